# revision 21
# baseline (speedup 1.0000x reference)
"""Trainium2 Bass kernel for nn_MultiHeadAttention (channel-attention transformer block).

Math (per batch b, with X* = reshape(*, [C, P]), P = 4096, C = 128, D = 512):
  Q = Xq @ (Wq/temp)^T, K = Xk @ Wk^T, V = Xv @ Wv^T            [C, D]
  per head h (8 heads, ld=64): A_h = softmax(Q_h K_h^T); O_h = A_h V_h
  O = silu(O); O = (O - mean)/(unbiased_std + eps)   (LN affine folded into fc)
  out_pre = (v + Wfc@ln_beta) + O @ (Wfc*ln_gamma)^T
  out = BatchNorm2d(out_pre)   (batch stats over (b,h,w), biased var)

Sharding: data-parallel over batch, 2 batches per core on 8 cores; BatchNorm
statistics combined with a tiny AllReduce ([128,4] per core).

Matmul dtype: float32r views of fp32 data (full PE rate at N>=256); switchable
to bf16 or plain f32 via BASS_MM_MODE.
"""

import os

import numpy as np

import concourse.mybir as mybir
import concourse.tile as tile
from concourse import bacc
from concourse.bass_utils import run_bass_kernel_spmd
from concourse.masks import make_identity

# ---- problem constants (hardcoded per contract) ----
B, C, HH, WW = 16, 128, 64, 64
P = HH * WW           # 4096
NH, LD = 8, 64
D = NH * LD           # 512
N_CORES = 8
BPC = B // N_CORES    # 2 batches per core
PCH2 = P // 256       # 16 double-chunks over the contraction dim
LN_EPS = 1e-6
BN_EPS = 1e-5
F32 = mybir.dt.float32
F32R = mybir.dt.float32r
BF16 = mybir.dt.bfloat16

MODE = os.environ.get("BASS_MM_MODE", "f32r")  # f32r | bf16 | f32
PHASE = os.environ.get("BASS_PHASE", "full")  # dma | proj | attn | fc | full

_BUILD_CACHE: dict = {}
LAST_RESULTS = None  # BassKernelResults of the most recent run (for profiling)


def _mm(ap):
    return ap


def _emit(ctx, nc, tc, io):
    act_dt = {"f32r": F32R, "bf16": BF16, "f32": F32}[MODE]    # matmul operand storage
    tr_dt = act_dt                                              # transpose data dtype
    AF = mybir.ActivationFunctionType
    ALU = mybir.AluOpType

    consts = ctx.enter_context(tc.tile_pool(name="consts", bufs=1))
    wpool = ctx.enter_context(tc.tile_pool(name="wpool", bufs=2))
    apool = ctx.enter_context(tc.tile_pool(name="apool", bufs=3))
    big = ctx.enter_context(tc.tile_pool(name="big", bufs=1))
    sb = ctx.enter_context(tc.tile_pool(name="sb", bufs=2))
    small = ctx.enter_context(tc.tile_pool(name="small", bufs=4))
    stat = ctx.enter_context(tc.tile_pool(name="stat", bufs=1))
    dram = ctx.enter_context(tc.tile_pool(name="dram", bufs=1, space="DRAM"))

    # identity for PE transposes; dummy transpose primes PE's view of the
    # identity-writer tick so later transposes carry a single sync wait
    ident_f = consts.tile([128, 128], F32, tag="identf", name="identf")
    make_identity(nc, ident_f)
    if MODE == "f32":
        ident = ident_f
    else:
        ident = consts.tile([128, 128], tr_dt, tag="ident", name="ident")
        nc.vector.tensor_copy(out=ident, in_=ident_f)

    bng = consts.tile([128, 1], F32, tag="bng", name="bng")
    bnb = consts.tile([128, 1], F32, tag="bnb", name="bnb")
    nc.sync.dma_start(out=bng, in_=io["bng"][:, :])
    nc.sync.dma_start(out=bnb, in_=io["bnb"][:, :])

    wfc_sb = []
    for dc in range(4):
        t = big.tile([128, P], act_dt, tag=f"wfc{dc}", name=f"wfc{dc}")
        nc.sync.dma_start(out=t, in_=io["wfc"][dc, :, :])
        wfc_sb.append(t)

    out_sb = []
    for b in range(BPC):
        t = big.tile([128, P], F32, tag=f"veff{b}", name=f"veff{b}")
        nc.sync.dma_start(out=t, in_=io["veff"][b, :, :])
        out_sb.append(t)

    if PHASE == "dma":
        for b in range(BPC):
            nc.gpsimd.dma_start(out=io["out"][b, :, :], in_=out_sb[b])
        return

    # ---- phase A: QKV projections, accumulating over the P=4096 contraction ----
    ps_proj = ctx_a = tc.tile_pool(name="ps_proj", bufs=1, space="PSUM")
    ps_proj = ctx_a.__enter__()
    warm = ps_proj.tile([128, 128], tr_dt, tag="warm", name="warm")
    nc.tensor.transpose(warm[:, :], ident[:, :], ident[:, :])
    Qp = [ps_proj.tile([128, D], F32, tag=f"Qp{b}", name=f"Qp{b}") for b in range(BPC)]
    Kp = [ps_proj.tile([128, D], F32, tag=f"Kp{b}", name=f"Kp{b}") for b in range(BPC)]
    Vp = [ps_proj.tile([128, D], F32, tag=f"Vp{b}", name=f"Vp{b}") for b in range(BPC)]

    for pc2 in range(PCH2):
        rows = slice(pc2 * 256, (pc2 + 1) * 256)
        wq_c = wpool.tile([128, 2, D], act_dt, tag="wq_c", name="wq_c")
        wk_c = wpool.tile([128, 2, D], act_dt, tag="wk_c", name="wk_c")
        wv_c = wpool.tile([128, 2, D], act_dt, tag="wv_c", name="wv_c")
        nc.sync.dma_start(out=wq_c, in_=io["wq"][rows, :].rearrange("(j p) d -> p j d", p=128))
        nc.sync.dma_start(out=wk_c, in_=io["wk"][rows, :].rearrange("(j p) d -> p j d", p=128))
        nc.sync.dma_start(out=wv_c, in_=io["wv"][rows, :].rearrange("(j p) d -> p j d", p=128))
        qc = apool.tile([128, BPC, 2, 128], act_dt, tag="qc", name="qc")
        kc = apool.tile([128, BPC, 2, 128], act_dt, tag="kc", name="kc")
        vc = apool.tile([128, BPC, 2, 128], act_dt, tag="vc", name="vc")
        for b in range(BPC):
            nc.sync.dma_start(out=qc[:, b, :, :], in_=io["qT"][b, rows, :].rearrange("(j p) c -> p j c", p=128))
            nc.sync.dma_start(out=kc[:, b, :, :], in_=io["kT"][b, rows, :].rearrange("(j p) c -> p j c", p=128))
            nc.sync.dma_start(out=vc[:, b, :, :], in_=io["vT"][b, rows, :].rearrange("(j p) c -> p j c", p=128))
        for j in range(2):
            st = pc2 == 0 and j == 0
            sp = pc2 == PCH2 - 1 and j == 1
            for b in range(BPC):
                nc.tensor.matmul(Qp[b][:, :], _mm(qc[:, b, j, :]), _mm(wq_c[:, j, :]), start=st, stop=sp)
                nc.tensor.matmul(Kp[b][:, :], _mm(kc[:, b, j, :]), _mm(wk_c[:, j, :]), start=st, stop=sp)
                nc.tensor.matmul(Vp[b][:, :], _mm(vc[:, b, j, :]), _mm(wv_c[:, j, :]), start=st, stop=sp)

    # ---- evacuate PSUM early: copies for both batches free all 6 proj banks ----
    qkv_sb = []
    for b in range(BPC):
        Q_sb = sb.tile([128, D], act_dt, tag="Q_sb", name="Q_sb")
        K_sb = sb.tile([128, D], act_dt, tag="K_sb", name="K_sb")
        V_sb = sb.tile([128, D], act_dt, tag="V_sb", name="V_sb")
        nc.vector.tensor_copy(out=Q_sb, in_=Qp[b][:, :])
        nc.vector.tensor_copy(out=K_sb, in_=Kp[b][:, :])
        nc.vector.tensor_copy(out=V_sb, in_=Vp[b][:, :])
        qkv_sb.append((Q_sb, K_sb, V_sb))
    ctx_a.__exit__(None, None, None)
    if PHASE == "proj":
        for b in range(BPC):
            nc.gpsimd.dma_start(out=io["out"][b, :, :], in_=out_sb[b])
        return
    ps_s = ctx.enter_context(tc.tile_pool(name="ps_s", bufs=2, space="PSUM"))
    ps_o = ctx.enter_context(tc.tile_pool(name="ps_o", bufs=2, space="PSUM"))
    ps_fc = ctx.enter_context(tc.tile_pool(name="ps_fc", bufs=2, space="PSUM"))

    # per-channel running sums for BN stats: cols [s1_b0, s1_b1, s2_b0, s2_b1]
    stats4 = stat.tile([128, 4], F32, tag="stats4", name="stats4")
    nc.vector.memset(stats4, 0.0)

    # ---- phases B-D per batch: attention, silu+LN, fc+residual ----
    for b in range(BPC):
        Q_sb, K_sb, V_sb = qkv_sb[b]

        QT_sb = sb.tile([128, D], act_dt, tag="QT_sb", name="QT_sb")
        KT_sb = sb.tile([128, D], act_dt, tag="KT_sb", name="KT_sb")
        for src, dst in ((Q_sb, QT_sb), (K_sb, KT_sb)):
            for dc in range(4):
                tp = ps_s.tile([128, 128], tr_dt, tag="stp", name="stp")
                nc.tensor.transpose(tp[:, :], _mm(src[:, dc * 128:(dc + 1) * 128]), ident[:, :])
                cp = tp[:, :].bitcast(F32) if MODE == "f32r" else tp[:, :]
                nc.vector.tensor_copy(out=dst[:, dc * 128:(dc + 1) * 128], in_=cp)

        Opsum = ps_o.tile([128, D], F32, tag="O", name="O")
        for h in range(NH):
            po = (h % 2) * 64
            fo = (h // 2) * 128
            S = ps_s.tile([128, 128], F32, tag="S", name="S")
            nc.tensor.matmul(
                S[:, :],
                _mm(QT_sb[po:po + 64, fo:fo + 128]),
                _mm(KT_sb[po:po + 64, fo:fo + 128]),
                start=True, stop=True,
            )
            negm = small.tile([128, 1], F32, tag="negm", name="negm")
            nc.vector.reduce_max(negm, S[:, :], axis=mybir.AxisListType.X, negate=True)
            lsum = small.tile([128, 1], F32, tag="lsum", name="lsum")
            nc.vector.memset(lsum, 0.0)
            e_f = sb.tile([128, 128], F32, tag="e_f", name="e_f")
            nc.scalar.activation(out=e_f, in_=S[:, :], func=AF.Exp,
                                 bias=negm, scale=1.0, accum_out=lsum)
            rs = small.tile([128, 1], F32, tag="rs", name="rs")
            nc.vector.reciprocal(rs, lsum)
            e = sb.tile([128, 128], act_dt, tag="e", name="e")
            nc.vector.tensor_scalar_mul(out=e, in0=e_f, scalar1=rs)
            tpa = ps_s.tile([128, 128], tr_dt, tag="stp", name="stp")
            nc.tensor.transpose(tpa[:, :], _mm(e[:, :]), ident[:, :])
            aT = sb.tile([128, 128], act_dt, tag="aT", name="aT")
            cp = tpa[:, :].bitcast(F32) if MODE == "f32r" else tpa[:, :]
            nc.vector.tensor_copy(out=aT, in_=cp)
            nc.tensor.matmul(
                Opsum[:, h * 64:(h + 1) * 64],
                _mm(aT[:, :]),
                _mm(V_sb[:, h * 64:(h + 1) * 64]),
                start=True, stop=True,
            )

        if PHASE == "attn":
            continue
        # silu + layernorm (affine folded into fc weights on host)
        sg = sb.tile([128, D], F32, tag="sg", name="sg")
        nc.scalar.activation(out=sg, in_=Opsum[:, :], func=AF.Sigmoid)
        Osw = sb.tile([128, D], F32, tag="Osw", name="Osw")
        nc.vector.tensor_mul(out=Osw, in0=Opsum[:, :], in1=sg)
        st6 = small.tile([128, 6], F32, tag="st6", name="st6")
        nc.vector.bn_stats(out=st6, in_=Osw)
        mv = small.tile([128, 2], F32, tag="mv", name="mv")
        nc.vector.bn_aggr(out=mv, in_=st6)
        sd = small.tile([128, 1], F32, tag="sd", name="sd")
        nc.scalar.activation(out=sd, in_=mv[:, 1:2], func=AF.Sqrt, scale=float(D) / (D - 1))
        nc.vector.tensor_scalar_add(out=sd, in0=sd, scalar1=LN_EPS)
        rstd = small.tile([128, 1], F32, tag="rstd", name="rstd")
        nc.vector.reciprocal(rstd, sd)
        xhat = sb.tile([128, D], act_dt, tag="xhat", name="xhat")
        nc.vector.tensor_scalar(out=xhat, in0=Osw, scalar1=mv[:, 0:1], scalar2=rstd,
                                op0=ALU.subtract, op1=ALU.mult)
        xT = sb.tile([128, D], act_dt, tag="xT", name="xT")
        for dc in range(4):
            tp = ps_s.tile([128, 128], tr_dt, tag="stp", name="stp")
            nc.tensor.transpose(tp[:, :], _mm(xhat[:, dc * 128:(dc + 1) * 128]), ident[:, :])
            cp = tp[:, :].bitcast(F32) if MODE == "f32r" else tp[:, :]
            nc.vector.tensor_copy(out=xT[:, dc * 128:(dc + 1) * 128], in_=cp)

        # fc + residual; fuse per-channel running sum for BN stats
        for pt in range(P // 512):
            O2 = ps_fc.tile([128, 512], F32, tag="O2", name="O2")
            for dc in range(4):
                nc.tensor.matmul(
                    O2[:, :],
                    _mm(xT[:, dc * 128:(dc + 1) * 128]),
                    _mm(wfc_sb[dc][:, pt * 512:(pt + 1) * 512]),
                    start=dc == 0, stop=dc == 3,
                )
            seg = out_sb[b][:, pt * 512:(pt + 1) * 512]
            nc.vector.tensor_add(out=seg, in0=seg, in1=O2[:, :])
        nc.vector.reduce_sum(stats4[:, b:b + 1], out_sb[b][:, :],
                             axis=mybir.AxisListType.X)

    if PHASE == "attn":
        for b in range(BPC):
            nc.gpsimd.dma_start(out=io["out"][b, :, :], in_=out_sb[b])
        return
    if PHASE == "fc":
        for b in range(BPC):
            nc.gpsimd.dma_start(out=io["out"][b, :, :], in_=out_sb[b])
        return

    # ---- phase E: BN stats (sum of squares), AllReduce, normalize ----
    junk = big.tile([128, P], F32, tag="junk", name="junk")
    for b in range(BPC):
        nc.scalar.activation(out=junk, in_=out_sb[b], func=AF.Square,
                             accum_out=stats4[:, 2 + b:3 + b])

    cin = dram.tile([128, 4], F32, tag="cin", name="cin")
    cout = dram.tile([128, 4], F32, tag="cout", name="cout")
    nc.gpsimd.dma_start(out=cin[:, :], in_=stats4)
    if os.environ.get("BASS_SKIP_COLL", "0") == "1":
        nc.gpsimd.dma_start(out=cout[:, :], in_=cin[:, :])
    else:
        nc.gpsimd.collective_compute(
            "AllReduce",
            ALU.add,
            replica_groups=[list(range(N_CORES))],
            ins=[cin.opt()],
            outs=[cout.opt()],
        )
    red = stat.tile([128, 4], F32, tag="red", name="red")
    nc.gpsimd.dma_start(out=red[:, :], in_=cout[:, :])

    inv_n = 1.0 / float(B * P)
    t1 = small.tile([128, 1], F32, tag="t1", name="t1")
    t2 = small.tile([128, 1], F32, tag="t2", name="t2")
    nc.vector.tensor_add(out=t1, in0=red[:, 0:1], in1=red[:, 1:2])
    nc.vector.tensor_add(out=t2, in0=red[:, 2:3], in1=red[:, 3:4])
    mean = small.tile([128, 1], F32, tag="mean", name="mean")
    nc.scalar.mul(out=mean, in_=t1, mul=inv_n)
    ex2 = small.tile([128, 1], F32, tag="ex2", name="ex2")
    nc.scalar.mul(out=ex2, in_=t2, mul=inv_n)
    msq = small.tile([128, 1], F32, tag="msq", name="msq")
    nc.vector.tensor_mul(out=msq, in0=mean, in1=mean)
    var = small.tile([128, 1], F32, tag="var", name="var")
    nc.vector.tensor_sub(out=var, in0=ex2, in1=msq)
    epsbn = consts.tile([128, 1], F32, tag="epsbn", name="epsbn")
    nc.vector.memset(epsbn, BN_EPS)
    sdv = small.tile([128, 1], F32, tag="sdv", name="sdv")
    nc.scalar.activation(out=sdv, in_=var, func=AF.Sqrt, bias=epsbn)
    invs = small.tile([128, 1], F32, tag="invs", name="invs")
    nc.vector.reciprocal(invs, sdv)
    scl = small.tile([128, 1], F32, tag="scl", name="scl")
    nc.vector.tensor_mul(out=scl, in0=bng, in1=invs)
    tmp = small.tile([128, 1], F32, tag="tmp", name="tmp")
    nc.vector.tensor_mul(out=tmp, in0=mean, in1=scl)
    shf = small.tile([128, 1], F32, tag="shf", name="shf")
    nc.vector.tensor_sub(out=shf, in0=bnb, in1=tmp)

    for b in range(BPC):
        nc.vector.tensor_scalar(out=out_sb[b], in0=out_sb[b], scalar1=scl, scalar2=shf,
                                op0=mybir.AluOpType.mult, op1=mybir.AluOpType.add)
        nc.gpsimd.dma_start(out=io["out"][b, :, :], in_=out_sb[b])


def _build():
    key = (MODE, PHASE, os.environ.get("BASS_SKIP_COLL", "0"))
    if key in _BUILD_CACHE:
        return _BUILD_CACHE[key]
    act_np = {"f32r": F32R, "bf16": BF16, "f32": F32}[MODE]
    nc = bacc.Bacc("TRN2", target_bir_lowering=False, debug=False, num_devices=N_CORES)
    io = {
        "qT": nc.dram_tensor("qT", [BPC, P, C], act_np, kind="ExternalInput").ap(),
        "kT": nc.dram_tensor("kT", [BPC, P, C], act_np, kind="ExternalInput").ap(),
        "vT": nc.dram_tensor("vT", [BPC, P, C], act_np, kind="ExternalInput").ap(),
        "veff": nc.dram_tensor("veff", [BPC, C, P], F32, kind="ExternalInput").ap(),
        "wq": nc.dram_tensor("wq", [P, D], act_np, kind="ExternalInput").ap(),
        "wk": nc.dram_tensor("wk", [P, D], act_np, kind="ExternalInput").ap(),
        "wv": nc.dram_tensor("wv", [P, D], act_np, kind="ExternalInput").ap(),
        "wfc": nc.dram_tensor("wfc", [4, 128, P], act_np, kind="ExternalInput").ap(),
        "bng": nc.dram_tensor("bng", [C, 1], F32, kind="ExternalInput").ap(),
        "bnb": nc.dram_tensor("bnb", [C, 1], F32, kind="ExternalInput").ap(),
        "out": nc.dram_tensor("out", [BPC, C, P], F32, kind="ExternalOutput").ap(),
    }
    from contextlib import ExitStack
    with tile.TileContext(nc) as tc, ExitStack() as ctx:
        _emit(ctx, nc, tc, io)
    nc.compile()
    _BUILD_CACHE[key] = nc
    return nc


def _np_cast(x):
    if MODE == "bf16":
        import ml_dtypes
        return np.asarray(x, np.float32).astype(ml_dtypes.bfloat16)
    return np.ascontiguousarray(np.asarray(x, np.float32))


def kernel(v, k, q, w_qs, w_ks, w_vs, w_fc, ln_gamma, ln_beta, temperature,
           bn_gamma, bn_beta, **_ignored):
    v = np.asarray(v, np.float32)
    k = np.asarray(k, np.float32)
    q = np.asarray(q, np.float32)
    w_qs = np.asarray(w_qs, np.float32)
    w_ks = np.asarray(w_ks, np.float32)
    w_vs = np.asarray(w_vs, np.float32)
    w_fc = np.asarray(w_fc, np.float32)
    ln_gamma = np.asarray(ln_gamma, np.float32)
    ln_beta = np.asarray(ln_beta, np.float32)
    temp = float(np.asarray(temperature))
    bn_gamma = np.asarray(bn_gamma, np.float32)
    bn_beta = np.asarray(bn_beta, np.float32)

    qf = q.reshape(B, C, P)
    kf = k.reshape(B, C, P)
    vf = v.reshape(B, C, P)
    qT = _np_cast(qf.transpose(0, 2, 1))
    kT = _np_cast(kf.transpose(0, 2, 1))
    vT = _np_cast(vf.transpose(0, 2, 1))
    wq = _np_cast((w_qs / temp).T)
    wk = _np_cast(w_ks.T)
    wv = _np_cast(w_vs.T)
    wfc_eff = _np_cast((w_fc * ln_gamma[None, :]).T.reshape(4, 128, P))
    bias_fc = (w_fc @ ln_beta).astype(np.float32)
    veff = np.ascontiguousarray(vf + bias_fc[None, None, :])
    bng = np.ascontiguousarray(bn_gamma.reshape(C, 1))
    bnb = np.ascontiguousarray(bn_beta.reshape(C, 1))

    nc = _build()
    in_maps = []
    for i in range(N_CORES):
        bs = slice(BPC * i, BPC * (i + 1))
        in_maps.append({
            "qT": qT[bs], "kT": kT[bs], "vT": vT[bs], "veff": veff[bs],
            "wq": wq, "wk": wk, "wv": wv, "wfc": wfc_eff,
            "bng": bng, "bnb": bnb,
        })
    res = run_bass_kernel_spmd(nc, in_maps, core_ids=list(range(N_CORES)))
    global LAST_RESULTS
    LAST_RESULTS = res
    out = np.concatenate([res.results[i]["out"] for i in range(N_CORES)], axis=0)
    return out.reshape(B, C, HH, WW).astype(np.float32)
